# revision 1
# baseline (speedup 1.0000x reference)
"""Trainium2 Bass kernel for nn_EquiCtsConvBase (equivariant continuous conv).

Math (per batch b, center m, field point n):
  rel = (field[n] - center[m]) / RADIUS;  r, theta = polar(rel)
  Bilinear grid-sample of kernel[(co,ci,y,x), theta_pad, r] decomposes into
  separable hats: Wx[j] = relu(1-|4r-0.5-j|), Wy[l] = relu(1-|iy-l|) with
  iy = 4*theta/pi + 4.5 and circular fold of 10 rows -> 8 theta bins.
  att = relu(1-|rel|^2)^3 * mask
  A[cell=(j,b), n, m] = Wx[j] * Wy8[b] * att
  G[f, cell, m] = sum_n feat[n, f] * A[cell, n, m]          (PE matmul 1)
  out[m, coy]   = sum_{cell,f} G * K2[cell, f, coy]         (PE matmul 2)
  out /= max(psi, tiny), psi[m] = sum_n att (ones-column of feat)

Device pipeline (fp16 elementwise, f32r PE front, fp16 matmuls):
  rho      = |f|^2 - 2 f.c + |c|^2   on PE (K=4 matmul per chunk) -> PSUM
  relx/y   = broadcast subtract on DVE (fp16)
  theta    via octant reduction: ONE divide (mn/mx on Pool), one Arctan,
           then arithmetic quadrant folds to iy directly (no copy_predicated)
  r        = mx / cos(phi0)  (second Pool divide)
  hats     as 1-|v-l| slots, abs via int16 bitmask, relu deferred into
           fused STT (max 0, * att); A-cells as 4 fused-8 TT products
  matmul1  fp16: 4 j-groups x 2 halves x 3 chunks into PSUM [17, 384]
  G        copied PSUM->SBUF with fp16 cast in [16, cell, m] layout, then
           4 SBUF->SBUF DMAs transpose to Gt[(f*8+b), j, m]
  matmul2  4 accumulating fp16 matmuls  out[m, coy] = Gt^T @ K2
  psi      row 16 of the att-cell matmul group; 1/psi via fast approx
           reciprocal, transposed to [96,1] by DMA, applied as TS scalar.

Sharding: 8 cores; core c handles batch b = c//4, centers m0 = (c%4)*96.
"""

import math
import numpy as np

RADIUS = 1.5
B, M, N = 2, 384, 384
CI = CO = 8
M_LOC = 96          # centers per core
NCH = 3             # n-chunks of 128 (N = 384)
NCELL = 32          # cell = j*8 + b  (j = radius 0..3, b = theta bin 0..7)
N_CORES = 8
PI = math.pi

CFG = dict()

_module_cache = {}

# fkin fp16 column layout
FEAT0 = 0          # 3 chunks * 33 (16 feat + 16 zero + mask; psi at row 32)
K20 = FEAT0 + 3 * 33   # 99: k2c, 4 j * 16 coy
FKW = K20 + 64     # 163

# bias f32 column layout: value of col i, then fp32 coordinates
BIAS_VALS = [0.0, 1.0, PI / 2] + [-float(l) for l in range(10)]  # 13 cols
BIAS_COL = {v: i for i, v in enumerate(BIAS_VALS)}
FX0 = 13           # fx per chunk: 3 cols
FY0 = 16
CX0 = 19           # cx per m: 96 cols
CY0 = 115
BIASW = CY0 + 96   # 211


def _build_module(cfg):
    import concourse.bass as bass
    import concourse.bacc as bacc
    import concourse.mybir as mybir
    from concourse import tile

    dt = mybir.dt
    Alu = mybir.AluOpType
    Act = mybir.ActivationFunctionType

    nc = bacc.Bacc("TRN2", target_bir_lowering=False, debug=False,
                   num_devices=N_CORES)

    f32 = dt.float32
    f32r = dt.float32r
    f16 = dt.float16
    i16 = dt.int16

    # ------------- DRAM I/O -------------
    ped = nc.dram_tensor("pein", [4, 480], f32, kind="ExternalInput").ap()
    biasd = nc.dram_tensor("bias", [128, BIASW], f32,
                           kind="ExternalInput").ap()
    fkd = nc.dram_tensor("fkin", [128, FKW], f16, kind="ExternalInput").ap()
    outd = nc.dram_tensor("out", [M_LOC, 16], f32, kind="ExternalOutput").ap()

    with tile.TileContext(nc) as tc:
        with tc.tile_pool(name="p", bufs=1) as pool, \
             tc.tile_pool(name="ps", bufs=1, space="PSUM") as psum:

            V, S, G = nc.vector, nc.scalar, nc.gpsimd

            # ---------- loads ----------
            fk_s = pool.tile([128, FKW], f16, tag="fk", name="fk_s")
            bias_s = pool.tile([128, BIASW], f32, tag="bias", name="bias_s")
            pe_s = pool.tile([4, 480], f32, tag="pein", name="pe_s")
            nc.sync.dma_start(bias_s[:], biasd[:])
            nc.scalar.dma_start(pe_s[:], ped[:])
            nc.gpsimd.dma_start(fk_s[:], fkd[:])

            def bAP(v, parts=128):
                return bias_s[0:parts, BIAS_COL[v]:BIAS_COL[v] + 1]

            # warm-up: force both act tables to load early, trig last
            warm = pool.tile([1, 1], f32, tag="warm", name="warm")
            V.memset(warm[:], 0.25)
            S.activation(warm[:], warm[:], Act.Arctan)
            S.activation(warm[:], warm[:], Act.Sin)

            def t16(tag, shape=None):
                return pool.tile(shape or [128, NCH, M_LOC], f16, tag=tag,
                                 name=tag)

            # ---------- PE front: rho -> PSUM ----------
            rho_p = psum.tile([128, NCH, M_LOC], f32, tag="rho", name="rho_p")
            for u in range(NCH):
                nc.tensor.matmul(
                    rho_p[:, u, :],
                    pe_s[0:4, u * 128:(u + 1) * 128],
                    pe_s[0:4, 384:480])

            # ---------- rel (fp32 coords -> fp16), fused over x|y ----------
            relxy = t16("relxy", [128, 2, NCH, M_LOC])
            f_b = bias_s[:, FX0:FX0 + 6, None].rearrange(
                "p (a u) o -> p a u o", a=2).to_broadcast(
                (128, 2, NCH, M_LOC))
            c_b = bias_s[:, None, CX0:CX0 + 192].rearrange(
                "p o (a m) -> p a o m", a=2).to_broadcast(
                (128, 2, NCH, M_LOC))
            V.tensor_tensor(relxy[:], f_b, c_b, Alu.subtract)
            relx, rely = relxy[:, 0], relxy[:, 1]

            # ---------- octant reduction ----------
            axy = t16("axy", [128, 2, NCH, M_LOC])
            sgxy = t16("sgxy", [128, 2, NCH, M_LOC])
            V.tensor_scalar(axy[:].bitcast(i16), relxy[:].bitcast(i16),
                            0x7FFF, None, Alu.bitwise_and)
            # sign as +-1.0 fp16: (bits & 0x8000) | 0x3C00
            V.tensor_scalar(sgxy[:].bitcast(i16), relxy[:].bitcast(i16),
                            -32768, 0x3C00, Alu.bitwise_and, Alu.bitwise_or)
            ax, ay = axy[:, 0], axy[:, 1]
            sgx, sgy = sgxy[:, 0], sgxy[:, 1]
            mn = t16("mn")
            mx = pool.tile([128, NCH, M_LOC], f32, tag="mx", name="mx")
            rmx = pool.tile([128, NCH, M_LOC], f32, tag="rmx", name="rmx")
            rsec = pool.tile([128, NCH, M_LOC], f32, tag="rsec", name="rsec")
            cosp = pool.tile([128, NCH, M_LOC], f32, tag="cosp", name="cosp")
            V.tensor_tensor(mn[:], ax, ay, Alu.min)
            V.tensor_tensor(mx[:], ax, ay, Alu.max)
            V.reciprocal_approx_fast(rmx[:], mx[:])
            ratio = t16("ratio")
            V.tensor_tensor(ratio[:], mn[:], rmx[:], Alu.mult)
            phi0 = t16("phi0")
            S.activation(phi0[:], ratio[:], Act.Arctan, bias=bAP(0.0))
            S.activation(cosp[:], phi0[:], Act.Sin, bias=bAP(PI / 2))
            V.reciprocal_approx_fast(rsec[:], cosp[:])
            rr = t16("rr")
            V.tensor_tensor(rr[:], mx[:], rsec[:], Alu.mult)

            # quadrant folds -> iy = 4*theta/pi + 4.5
            swp = t16("swp"); s1 = t16("s1"); tq = t16("tq"); q1 = t16("q1")
            t2a = t16("t2a"); q2m = t16("q2m"); u1t = t16("u1t")
            w2 = t16("w2"); iy = t16("iy")
            V.tensor_tensor(swp[:], ay, ax, Alu.is_gt)
            G.tensor_scalar(s1[:], swp[:], -2.0, 1.0, Alu.mult, Alu.add)
            G.tensor_scalar(w2[:], sgy, 2.0, 4.5, Alu.mult, Alu.add)
            V.tensor_tensor(tq[:], phi0[:], s1[:], Alu.mult)
            V.scalar_tensor_tensor(q1[:], swp[:], PI / 2, tq[:],
                                   Alu.mult, Alu.add)
            V.tensor_scalar(t2a[:], q1[:], -PI / 2, None, Alu.add)
            V.tensor_tensor(q2m[:], t2a[:], sgx, Alu.mult)
            V.tensor_tensor(u1t[:], sgy, q2m[:], Alu.mult)
            V.scalar_tensor_tensor(iy[:], u1t[:], 4.0 / PI, w2[:],
                                   Alu.mult, Alu.add)

            # ---------- att ----------
            au = t16("au"); au2 = t16("au2"); att = t16("att")
            S.activation(au[:], rho_p[:], Act.Relu, bias=bAP(1.0), scale=-1.0)
            S.activation(au2[:], au[:], Act.Square, bias=bAP(0.0))
            G.tensor_tensor(att[:], au2[:], au[:], Alu.mult)

            # a_t: 32 product cells + att cell
            a_t = pool.tile([128, NCELL + 1, NCH, M_LOC], f16, tag="a_t",
                            name="a_t")
            V.tensor_copy(a_t[:, NCELL], att[:])

            # ---------- matmul1: psi group first ----------
            def feat_ap(u):
                return fk_s[:, FEAT0 + 33 * u:FEAT0 + 33 * (u + 1)]

            gpsi = psum.tile([33, M_LOC], f32, tag="gpsi", name="gpsi")
            for u in range(NCH):
                nc.tensor.matmul(gpsi[:], feat_ap(u), a_t[:, NCELL, u, :],
                                 start=(u == 0), stop=(u == NCH - 1))

            # ---------- hat slots ----------
            # e12: rows 0..7 = |iy-(b+1)| (theta bins), rows 8..11 = |v4-j|
            v4 = t16("v4")
            V.tensor_scalar(v4[:], rr[:], 4.0, -0.5, Alu.mult, Alu.add)
            e12 = t16("e12", [128, 12, NCH, M_LOC])
            e9m = t16("e9m")

            def abs_slot(eng, dst, src, lf):
                if eng is S:
                    S.activation(dst, src, Act.Abs, bias=bAP(-float(lf)))
                else:
                    V.tensor_scalar(dst, src, -float(lf), None, Alu.add)
                    V.tensor_scalar(dst.bitcast(i16), dst.bitcast(i16),
                                    0x7FFF, None, Alu.bitwise_and)

            # Wx slots (v4 ready first): ACT
            for j in range(4):
                abs_slot(S, e12[:, 8 + j], v4[:], j)
            # Wy slots: DVE takes fold-involved + l=5..8, ACT takes l=2..4
            V.tensor_scalar(e9m[:], iy[:], -1.0, 9.0, Alu.mult, Alu.add)
            abs_slot(V, e12[:, 0], iy[:], 1)
            abs_slot(V, e12[:, 7], iy[:], 8)
            for l in (2, 3, 4):
                abs_slot(S, e12[:, l - 1], iy[:], l)
            for l in (5, 6, 7):
                abs_slot(V, e12[:, l - 1], iy[:], l)
            # circular fold: |iy-9| = 9-iy, |iy-0| = iy (iy in [0.5, 8.5])
            V.tensor_tensor(e12[:, 0], e12[:, 0], e9m[:], Alu.min)
            V.tensor_tensor(e12[:, 7], e12[:, 7], iy[:], Alu.min)
            # hats: w12 = relu(1 - e12); wxa = wx * att
            pre12 = t16("pre12", [128, 12, NCH, M_LOC])
            w12 = t16("w12", [128, 12, NCH, M_LOC])
            wxa = t16("wxa", [128, 4, NCH, M_LOC])
            V.tensor_scalar(pre12[:], e12[:], -1.0, 1.0, Alu.mult, Alu.add)
            V.tensor_scalar(w12[:], pre12[:], 0.0, None, Alu.max)
            wya = w12[:, 0:8]
            att_b4 = att[:, None].to_broadcast((128, 4, NCH, M_LOC))
            V.tensor_tensor(wxa[:], w12[:, 8:12], att_b4, Alu.mult)

            # ---------- PE warm-up before matmul1 (p-state ramp) ----------
            warm_ps = psum.tile([64, NCH * M_LOC], f32, tag="warmps",
                                name="warm_ps")
            for _ in range(8):
                nc.tensor.matmul(warm_ps[:], w12[:, 0, 0, 0:64],
                                 w12[:, 0, :, :])

            # ---------- A cells: cell = j*8 + b ----------
            def wx_b(j, nb):
                return wxa[:, j, None].to_broadcast((128, nb, NCH, M_LOC))

            for j in (0, 1):
                V.tensor_tensor(a_t[:, 8 * j:8 * j + 8], wx_b(j, 8), wya,
                                Alu.mult)
            G.tensor_tensor(a_t[:, 24:28], wx_b(3, 4), wya[:, 0:4], Alu.mult)
            V.tensor_tensor(a_t[:, 28:32], wx_b(3, 4), wya[:, 4:8], Alu.mult)
            V.tensor_tensor(a_t[:, 16:24], wx_b(2, 8), wya, Alu.mult)

            # ---------- matmul1 j-groups + G transpose ----------
            gs_s = pool.tile([16, NCELL, M_LOC], f16, tag="gs", name="gs_s")
            cpeng = {0: S, 1: V, 2: S, 3: S, 4: S, 5: V, 6: S, 7: S}
            gh = [psum.tile([33, 4 * M_LOC], f32, tag=f"gh{i}",
                            name=f"gh{i}") for i in range(4)]
            for gi, j in enumerate((0, 1, 3, 2)):
                for h in range(2):
                    g = gh[(2 * gi + h) % 4]
                    c0 = 8 * j + 4 * h
                    for u in range(NCH):
                        nc.tensor.matmul(g[:], feat_ap(u),
                                         a_t[:, c0:c0 + 4, u, :],
                                         start=(u == 0), stop=(u == NCH - 1))
                    eng = cpeng[2 * gi + h]
                    if eng is S:
                        S.activation(gs_s[:, c0:c0 + 4], g[0:16], Act.Copy)
                    else:
                        eng.tensor_copy(gs_s[:, c0:c0 + 4], g[0:16])

            # Gt[(f*8+b), j, m] <- gs_s[f, j*8+b, m]
            gt = pool.tile([128, 4, M_LOC], f16, tag="gt", name="gt")
            for j in range(4):
                nc.sync.dma_start(gt[:, j], gs_s[:, 8 * j:8 * j + 8, :])

            # ---------- psi -> 1/psi -> [96, 1] ----------
            psir = pool.tile([1, M_LOC], f32, tag="psir", name="psir")
            V.tensor_scalar(psir[:], gpsi[32:33, :], 1e-35, None, Alu.max)
            V.reciprocal_approx_fast(psir[:], psir[:])
            psit = pool.tile([M_LOC, 1], f32, tag="psit", name="psit")
            nc.sync.dma_start(psit[:, 0:1], psir[0:1, :])

            # ---------- matmul2 ----------
            o2t = psum.tile([M_LOC, 16], f32, tag="o2t", name="o2t")
            for q in range(4):
                nc.tensor.matmul(o2t[:], gt[:, q, :],
                                 fk_s[:, K20 + 16 * q:K20 + 16 * (q + 1)],
                                 start=(q == 0), stop=(q == 3))

            # ---------- scale by 1/psi, store ----------
            out_s = pool.tile([M_LOC, 16], f32, tag="outs", name="out_s")
            V.tensor_scalar(out_s[:], o2t[:], psit[:, 0:1], None, Alu.mult)
            nc.sync.dma_start(outd[:], out_s[:])

    nc.compile()
    return nc


def get_module(cfg=None):
    cfg = dict(CFG, **(cfg or {}))
    key = tuple(sorted((k, str(v)) for k, v in cfg.items()))
    if key not in _module_cache:
        _module_cache[key] = _build_module(cfg)
    return _module_cache[key]


def make_in_maps(field, center, field_feat, field_mask, kernel, cfg=None):
    """Host-side shard + layout prep. Returns list of 8 in_maps."""
    field = np.asarray(field, np.float32)
    center = np.asarray(center, np.float32)
    feat = np.asarray(field_feat, np.float32)
    mask = np.asarray(field_mask, np.float32)
    ker = np.asarray(kernel, np.float32)

    # kk[cell=(th*4+r), f=(ci,x), coy=(co,y)]
    kk = ker.transpose(3, 2, 1, 5, 0, 4).reshape(NCELL, 16, 16)
    # k2c[p=(f*8+b), j, coy] = kk[b*4+j, f, coy]
    k2c = np.zeros((128, 4, 16), np.float32)
    for bth in range(8):
        for j in range(4):
            for f in range(16):
                k2c[f * 8 + bth, j] = kk[bth * 4 + j, f]

    in_maps = []
    for c in range(N_CORES):
        b, blk = divmod(c, 4)
        m0 = blk * M_LOC
        cx = center[b, m0:m0 + M_LOC, 0] / RADIUS   # [96]
        cy = center[b, m0:m0 + M_LOC, 1] / RADIUS
        fx = (field[b, :, 0] / RADIUS).reshape(NCH, 128)  # [3, 128]
        fy = (field[b, :, 1] / RADIUS).reshape(NCH, 128)
        ffsq = fx * fx + fy * fy
        ccsq = cx * cx + cy * cy

        pein = np.zeros((4, 480), np.float32)
        pein[0, 0:384] = ffsq.reshape(-1)
        pein[1, 0:384] = fx.reshape(-1)
        pein[2, 0:384] = fy.reshape(-1)
        pein[3, 0:384] = 1.0
        pein[0, 384:480] = 1.0
        pein[1, 384:480] = -2.0 * cx
        pein[2, 384:480] = -2.0 * cy
        pein[3, 384:480] = ccsq

        biasf = np.zeros((128, BIASW), np.float32)
        biasf[:, 0:len(BIAS_VALS)] = np.array(BIAS_VALS, np.float32)
        biasf[:, FX0:FX0 + 3] = fx.T
        biasf[:, FY0:FY0 + 3] = fy.T
        biasf[:, CX0:CX0 + 96] = cx
        biasf[:, CY0:CY0 + 96] = cy

        fkin = np.zeros((128, FKW), np.float32)
        fm = feat[b].reshape(N, 16) * mask[b]
        fcols = np.concatenate([fm, np.zeros((N, 16), np.float32), mask[b]],
                               axis=1)                      # [N, 33]
        fkin[:, FEAT0:FEAT0 + 99] = (
            fcols.reshape(NCH, 128, 33).transpose(1, 0, 2).reshape(128, 99))
        fkin[:, K20:K20 + 64] = k2c.reshape(128, 64)

        in_maps.append({
            "pein": pein,
            "bias": biasf,
            "fkin": fkin.astype(np.float16),
        })
    return in_maps


def unshard(results):
    out = np.zeros((B, M, CO, 2), np.float32)
    for c in range(N_CORES):
        b, blk = divmod(c, 4)
        m0 = blk * M_LOC
        out[b, m0:m0 + M_LOC] = results[c]["out"].reshape(M_LOC, CO, 2)
    return out


def kernel(field, center, field_feat, field_mask, kernel):
    from concourse.bass_utils import run_bass_kernel_spmd
    nc = get_module()
    in_maps = make_in_maps(field, center, field_feat, field_mask, kernel)
    res = run_bass_kernel_spmd(nc, in_maps, core_ids=list(range(N_CORES)))
    return unshard(res.results)

